# revision 13
# baseline (speedup 1.0000x reference)
"""Trainium2 Bass kernel for nn_DenoiseKTNet (4-layer dense transformer,
B=32, S=512, D=512, FF=2048, H=8, causal self-attention with Q=K).

Sharding: data-parallel over batch across 8 NeuronCores (4 items/core),
weights replicated, no collectives. Activations live feature-major
([features(part), chunk, tokens]) so every projection is a plain
lhsT.T @ rhs with a 512-wide moving operand.

v2 engine-rebalanced layout: all activations/weights fp16 (2-byte DVE
fast path + fp16 matmul moving operands), softmax exp + PSUM->SBUF bias
copies on the scalar engine, causal mask multiplies + z^2 squares on
GPSIMD, attention denominators broadcast raw via PE then reciprocal'd
as one [128,S] DVE op, LN finalize as 2x/4x-mode DVE ops on fp16
broadcast tiles, and a single activation-table set (no reload thrash).
"""
import numpy as np
import ml_dtypes

import concourse.bacc as bacc
import concourse.mybir as mybir
import concourse.tile as tile
from concourse.bass_utils import run_bass_kernel_spmd

P = 128
S = 512
D = 512
FF = 2048
H = 8
DK = 64
L = 4
B = 32
NCORES = 8
BLOC = B // NCORES
EPS = 1e-5          # layernorm eps (matches reference)
DEPS = 1e-4         # softmax denominator guard (query row 0 fully masked);
                    # real denominators are >= exp(-|score|) >> 1e-4

import os
FP32 = mybir.dt.float32
FP16 = (mybir.dt.bfloat16 if os.environ.get("K_ADT", "fp16") == "bf16"
        else mybir.dt.float16)   # 2-byte activation/weight dtype
NP16 = (ml_dtypes.bfloat16 if os.environ.get("K_ADT", "fp16") == "bf16"
        else np.float16)
MASK_POOL = os.environ.get("K_MASK_POOL", "1") == "1"
ZSQ_POOL = os.environ.get("K_ZSQ_POOL", "1") == "1"
PIN_ACT = os.environ.get("K_PIN_ACT", "1") == "1"
# filler-pull budget (ns of PE work) at each attention gap point:
# [hp0, hp1, hp2, hp3] ends + pre-loop
_pb = int(os.environ.get("K_PULL", "3500"))
PULL_HP = [_pb, _pb, _pb, _pb]
PULL_PRE = _pb
AF = mybir.ActivationFunctionType
MUL = mybir.AluOpType.mult
ADD = mybir.AluOpType.add
SUB = mybir.AluOpType.subtract
MAX = mybir.AluOpType.max

# column-pack indices inside the "cols" [128, L, 48] fp32 input
CI_BK, CI_BO, CI_BVWO, CI_B1, CI_B2, CI_G1, CI_BT1, CI_G2, CI_BT2 = (
    0, 4, 8, 12, 28, 32, 36, 40, 44)

_ACT_SET = "natural_log_exp_and_others"
_ACT_FUNCS = (AF.Identity, AF.Exp, AF.Ln, AF.Relu, AF.Square, AF.Copy)


def _pin_act_tables(arch):
    """Make every activation func we use first-match in one table set so
    the compiler emits a single LoadActFuncSet instead of thrashing
    between per-func minimal sets. Only claimed coverage of other sets is
    shrunk; set ids remain valid indices into act_info.json."""
    import concourse.hw_specs as hws
    try:
        tabs = hws.get_activation_tables(arch)
    except Exception:
        return
    if _ACT_SET not in tabs:
        return
    ours = set(_ACT_FUNCS)
    if not ours <= tabs[_ACT_SET]:
        return
    for name, s in tabs.items():
        if name != _ACT_SET:
            s -= ours


def build_program(cexp, n_layers=L, n_batch=BLOC, loop_reps=None,
                  static_weights=False, skip_attn=False, skip_ffn=False):
    nc = bacc.Bacc("TRN2", target_bir_lowering=False)
    if PIN_ACT:
        _pin_act_tables(nc.m.arch)

    x_dr = nc.dram_tensor("x", [n_batch, D, S], FP16, kind="ExternalInput")
    y_dr = nc.dram_tensor("y", [n_batch, D, S], FP16, kind="ExternalInput")
    wk_dr = nc.dram_tensor("wk", [n_layers, D, D], FP16, kind="ExternalInput")
    wv_dr = nc.dram_tensor("wv", [n_layers, D, D], FP16, kind="ExternalInput")
    wo_dr = nc.dram_tensor("wo", [n_layers, D, D], FP16, kind="ExternalInput")
    w1_dr = nc.dram_tensor("w1", [n_layers, D, FF], FP16, kind="ExternalInput")
    w2_dr = nc.dram_tensor("w2", [n_layers, FF, D], FP16, kind="ExternalInput")
    cols_dr = nc.dram_tensor("cols", [P, n_layers, 48], FP32, kind="ExternalInput")
    mask_dr = nc.dram_tensor("maskbf", [P, P], FP16, kind="ExternalInput")
    colh_dr = nc.dram_tensor("colh", [P, 2], FP16, kind="ExternalInput")
    rowr_dr = nc.dram_tensor("rowr", [1, 3, P], FP16, kind="ExternalInput")
    epsc_dr = nc.dram_tensor("epsc", [1, 1], FP32, kind="ExternalInput")
    out_dr = nc.dram_tensor("out", [n_batch, D, S], FP16, kind="ExternalOutput")

    with tile.TileContext(nc) as tc:
        with (
            tc.tile_pool(name="cpool", bufs=1) as cpool,      # consts
            tc.tile_pool(name="xpool", bufs=1) as xpool,      # resident streams
            tc.tile_pool(name="wpool", bufs=1) as wpool,      # layer weights
            tc.tile_pool(name="apool", bufs=1) as apool,      # block temps
            tc.tile_pool(name="epool", bufs=2) as epool,      # exp'd scores
            tc.tile_pool(name="rpool", bufs=1) as rpool,      # small rows
            tc.tile_pool(name="pp1", bufs=2, space="PSUM") as pp1,
            tc.tile_pool(name="pp2", bufs=2, space="PSUM") as pp2,
            tc.tile_pool(name="pp3", bufs=4, space="PSUM") as pp3,
        ):
            # ---- consts ----
            cols = cpool.tile([P, n_layers, 48], FP32)
            nc.sync.dma_start(cols, cols_dr.ap())
            mask = cpool.tile([P, P], FP16)
            nc.sync.dma_start(mask, mask_dr.ap())
            colh = cpool.tile([P, 2], FP16)           # [:,0]=-1/512, [:,1]=+1/512
            nc.sync.dma_start(colh, colh_dr.ap())
            rowr = cpool.tile([1, 3, P], FP16)        # ones / selA / selB
            nc.sync.dma_start(rowr, rowr_dr.ap())
            epsc = cpool.tile([1, 1], FP32)
            nc.sync.dma_start(epsc, epsc_dr.ap())

            # ---- resident activation streams ----
            xs, ys = [], []
            for b in range(n_batch):
                xb = xpool.tile([P, 4, S], FP16, name=f"xs{b}", tag=f"xs{b}")
                nc.sync.dma_start(xb, x_dr.ap()[b].rearrange("(c p) s -> p c s", p=P))
                xs.append(xb)
                yb = xpool.tile([P, 4, S], FP16, name=f"ys{b}", tag=f"ys{b}")
                nc.sync.dma_start(yb, y_dr.ap()[b].rearrange("(c p) s -> p c s", p=P))
                ys.append(yb)
            # v tiles (double-buffered by item parity), ones row per head at
            # dim DK for the fused softmax-denominator
            vts = []
            for i in range(2):
                vt = xpool.tile([P, 4, H, DK + 1], FP16, name=f"vones{i}",
                                tag=f"vones{i}")
                nc.gpsimd.memset(vt[:, :, :, DK:DK + 1], 1.0)
                vts.append(vt)

            def layer_norm_stats(z_sb):
                """LN stats for z [128,4,512] fp16; returns (rows, rstd)
                where rows = -mean and rstd = 1/sqrt(var+eps), both [1,S]
                fp16 (broadcast-matmul moving operands)."""
                zeng = nc.gpsimd if ZSQ_POOL else nc.vector
                zsqs = []
                for ki in range(4):
                    zsq = apool.tile([P, S], FP16, tag="zsq", bufs=4,
                                     name=f"zsq{ki}")
                    zeng.tensor_mul(zsq, z_sb[:, ki, :], z_sb[:, ki, :])
                    zsqs.append(zsq)
                mu_ps = pp1.tile([1, S], FP32, tag="ps1")
                for ki in range(4):
                    nc.tensor.matmul(mu_ps, colh[:, 0:1], z_sb[:, ki, :],
                                     start=(ki == 0), stop=(ki == 3))
                msq_ps = pp1.tile([1, S], FP32, tag="ps1")
                for ki in range(4):
                    nc.tensor.matmul(msq_ps, colh[:, 1:2], zsqs[ki],
                                     start=(ki == 0), stop=(ki == 3))
                rows = rpool.tile([1, S], FP16, tag="rows", bufs=3)   # -mu
                nc.scalar.activation(rows, mu_ps, AF.Identity)
                musq = rpool.tile([1, S], FP32, tag="musq", bufs=3)   # mu^2
                nc.scalar.activation(musq, mu_ps, AF.Square)
                var = rpool.tile([1, S], FP32, tag="rowtmp", bufs=2)
                nc.vector.tensor_tensor(var, msq_ps, musq, SUB)
                lnv = rpool.tile([1, S], FP32, tag="rowtmp", bufs=2)
                nc.scalar.activation(lnv, var, AF.Ln, bias=epsc)
                rstd = rpool.tile([1, S], FP16, tag="rstd", bufs=3)
                nc.scalar.activation(rstd, lnv, AF.Exp, scale=-0.5)
                return rows, rstd

            def make_ln_finalize(z_sb, out_sb, l, gi, bi, rows, rstd,
                                 pool=None):
                def fin():
                    mu_bc = pp2.tile([P, S], FP32, tag="ps2")
                    nc.tensor.matmul(mu_bc, rowr[0:1, 0, :], rows,
                                     start=True, stop=True)
                    rstd_bc = pp2.tile([P, S], FP32, tag="ps2")
                    nc.tensor.matmul(rstd_bc, rowr[0:1, 0, :], rstd,
                                     start=True, stop=True)
                    muB = apool.tile([P, S], FP16, tag="muB", bufs=3)
                    nc.scalar.activation(muB, mu_bc, AF.Identity)
                    rstdB = apool.tile([P, S], FP16, tag="rstdB", bufs=3)
                    nc.scalar.activation(rstdB, rstd_bc, AF.Identity)
                    for eo in range(4):
                        t = apool.tile([P, S], FP16, tag="t", bufs=2)
                        nc.vector.tensor_tensor(t, z_sb[:, eo, :], muB, ADD)
                        nc.vector.tensor_tensor(t, t, rstdB, MUL)
                        nc.vector.tensor_scalar(
                            out=out_sb[:, eo, :], in0=t,
                            scalar1=cols[:, l, gi + eo:gi + eo + 1],
                            scalar2=cols[:, l, bi + eo:bi + eo + 1],
                            op0=MUL, op1=ADD)
                return fin

            def emit_qkproj(l, b):
                """qk projection (feature-major out, fp16, bias on ACT)."""
                qk = apool.tile([P, 4, S], FP16, tag="qk", bufs=2)
                for eo in range(4):
                    ps = pp1.tile([P, S], FP32, tag="ps1")
                    for ki in range(4):
                        nc.tensor.matmul(ps, wk[:, ki, P * eo:P * (eo + 1)],
                                         xs[b][:, ki, :],
                                         start=(ki == 0), stop=(ki == 3))
                    nc.scalar.activation(qk[:, eo, :], ps, AF.Identity,
                                         bias=cols[:, l, CI_BK + eo:CI_BK + eo + 1])
                return qk

            def emit_vproj(b, vt):
                """v projection (token-major out, fp16, per-head 65-column
                groups with the fused-denominator ones row at dim DK)."""
                for sc in range(4):
                    ps = pp1.tile([P, H, DK], FP32, tag="ps1")
                    for ki in range(4):
                        nc.tensor.matmul(ps, ys[b][:, ki, P * sc:P * (sc + 1)],
                                         wv[:, ki, :],
                                         start=(ki == 0), stop=(ki == 3))
                    nc.vector.tensor_copy(vt[:, sc, :, 0:DK], ps)

            def emit_attn(l, b, vt, qk, hooks=(), pull=None):
                """Per-head attention with fused denominator; normalize of
                head-pair hp deferred past hp+1's matmuls (skew) to hide the
                reciprocal latency. hooks[hp] (if set) is emitted at the end
                of head-pair hp — used to slot the next item's projection
                matmuls into this item's attention chains. pull(ns) draws
                FFN filler units into the exp-latency gaps. Returns attnT."""
                attnT = apool.tile([P, 4, S], FP16, tag="attnT")

                def make_norm(hp, ats, dpr):
                    def norm():
                        bb_ps = pp2.tile([P, S], FP32, tag="ps2")
                        nc.tensor.matmul(bb_ps, rowr[0:1, 1, :],
                                         dpr[0:1, 0, :], start=True, stop=False)
                        nc.tensor.matmul(bb_ps, rowr[0:1, 2, :],
                                         dpr[0:1, 1, :], start=False, stop=True)
                        bbi = apool.tile([P, S], FP32, tag="bbi", bufs=2)
                        nc.vector.reciprocal_approx_fast(out=bbi, in_=bb_ps)
                        for q in range(2):
                            nc.vector.tensor_tensor(
                                attnT[DK * q:DK * (q + 1), hp, :],
                                ats[q][0:DK, :],
                                bbi[DK * q:DK * (q + 1), :], MUL)
                    return norm

                norm_pend = None
                for hp in range(4):
                    et = epool.tile([P, 4, 2, S], FP16, tag="E")
                    # both heads' scores first: head q=1's score matmuls give
                    # the PE work while head q=0's exp chain drains on the
                    # scalar engine
                    for q in range(2):
                        base = DK * q
                        for c in range(4):
                            sc_ps = pp2.tile([P, S], FP32, tag="ps2")
                            nc.tensor.matmul(
                                sc_ps[:, P * c:],
                                qk[base:base + DK, hp, P * c:P * (c + 1)],
                                qk[base:base + DK, hp, P * c:],
                                start=True, stop=True)
                            nc.scalar.activation(et[:, c, q, P * c:],
                                                 sc_ps[:, P * c:],
                                                 AF.Exp, scale=cexp)
                            meng = nc.gpsimd if MASK_POOL else nc.vector
                            meng.tensor_mul(
                                et[:, c, q, P * c:P * (c + 1)],
                                et[:, c, q, P * c:P * (c + 1)],
                                mask)
                    if hp < len(hooks) and hooks[hp] is not None:
                        # slot independent work (next item's projections, LN
                        # finalize) into the exp-latency gap before attn@V
                        hooks[hp]()
                        hooks[hp] = None
                    if pull is not None:
                        pull(PULL_HP[hp])
                    ats = []
                    # raw denominators (row DK of each at tile) -> SBUF fp16
                    # with the row-0 guard folded in as max(d, DEPS); emitted
                    # per-q right after its at-matmuls so the reciprocal
                    # chain starts while the other head still multiplies
                    dpr = rpool.tile([1, 2, S], FP16, tag="dpr", bufs=2)
                    for q in range(2):
                        h = 2 * hp + q
                        at = pp3.tile([DK + 1, S], FP32, tag="ps3")
                        for c in range(4):
                            nc.tensor.matmul(at[:, P * c:], vt[:, c, h, :],
                                             et[:, c, q, P * c:],
                                             start=(c == 0), stop=(c == 3),
                                             skip_group_check=(c > 0))
                        ats.append(at)
                        nc.vector.tensor_scalar(
                            out=dpr[0:1, q, :], in0=at[DK:DK + 1, :],
                            scalar1=DEPS, scalar2=None, op0=MAX)
                    if norm_pend is not None:
                        norm_pend()
                    norm_pend = make_norm(hp, ats, dpr)
                norm_pend()
                return attnT

            import contextlib
            if static_weights:
                # timing experiment: load layer-0 weights once, reuse for all
                # layers (wrong math, DMA-free steady state)
                wk = wpool.tile([P, 4, D], FP16, tag="wk")
                nc.sync.dma_start(wk, wk_dr.ap()[0].rearrange("(c p) n -> p c n", p=P))
                wv = wpool.tile([P, 4, D], FP16, tag="wv")
                nc.sync.dma_start(wv, wv_dr.ap()[0].rearrange("(c p) n -> p c n", p=P))
                wo = wpool.tile([P, 4, D], FP16, tag="wo")
                nc.sync.dma_start(wo, wo_dr.ap()[0].rearrange("(c p) n -> p c n", p=P))
                w1 = wpool.tile([P, 4, FF], FP16, tag="w1")
                nc.sync.dma_start(w1, w1_dr.ap()[0].rearrange("(c p) n -> p c n", p=P))
                w2 = wpool.tile([P, 16, D], FP16, tag="w2")
                nc.sync.dma_start(w2, w2_dr.ap()[0].rearrange("(c p) n -> p c n", p=P))
            from collections import deque
            loop_cm = tc.For_i(0, loop_reps, 1) if loop_reps else contextlib.nullcontext()
            with loop_cm:
              fins = []      # pending (target_b, finalize) — emitted late
              filler = deque()   # (pe_cost_ns, closure) FFN work units

              def flush_fin(target=None):
                  if target is None:
                      while fins:
                          fins.pop(0)[1]()
                  else:
                      keep = []
                      for tb, fn in fins:
                          if tb == target:
                              fn()
                          else:
                              keep.append((tb, fn))
                      fins[:] = keep

              def pull(budget):
                  while filler and budget > 0:
                      c, f = filler.popleft()
                      f()
                      budget -= c

              def drain():
                  while filler:
                      filler.popleft()[1]()

              def push_ffn(l, b, w1t, w2t):
                  """FFN(l, b) as filler units: 16 f1 chunks (4 MMs + ACT
                  relu), 4 f2 chunks (16 MMs + residual STT), then LN2 stats.
                  Unit 0 flushes pending finalizes (xs[b] must be final)."""
                  st = {}

                  def u_f1(fo):
                      def f(w1t=w1t):
                          if fo == 0:
                              flush_fin(b)
                              st["h"] = apool.tile([P, 16, S], FP16,
                                                   name=f"h{l}_{b}",
                                                   tag="h", bufs=2)
                          ps = pp1.tile([P, S], FP32, tag="ps1")
                          for ki in range(4):
                              nc.tensor.matmul(ps,
                                               w1t[:, ki, P * fo:P * (fo + 1)],
                                               xs[b][:, ki, :],
                                               start=(ki == 0), stop=(ki == 3))
                          nc.scalar.activation(
                              st["h"][:, fo, :], ps, AF.Relu,
                              bias=cols[:, l, CI_B1 + fo:CI_B1 + fo + 1])
                      return (900, f)

                  def u_f2(eo):
                      def f(w2t=w2t):
                          if eo == 0:
                              st["z2"] = apool.tile([P, 4, S], FP16,
                                                    name=f"z2_{l}_{b}",
                                                    tag="z", bufs=3)
                          ps = pp1.tile([P, S], FP32, tag="ps1")
                          for ki in range(16):
                              nc.tensor.matmul(ps,
                                               w2t[:, ki, P * eo:P * (eo + 1)],
                                               st["h"][:, ki, :],
                                               start=(ki == 0), stop=(ki == 15))
                          nc.vector.scalar_tensor_tensor(
                              out=st["z2"][:, eo, :], in0=ps,
                              scalar=cols[:, l, CI_B2 + eo:CI_B2 + eo + 1],
                              in1=xs[b][:, eo, :],
                              op0=ADD, op1=ADD)
                      return (3500, f)

                  def u_st():
                      def f():
                          rows, rstd = layer_norm_stats(st["z2"])
                          fins.append((b, make_ln_finalize(
                              st["z2"], xs[b], l, CI_G2, CI_BT2,
                              rows, rstd, pp3)))
                      return (1800, f)

                  for fo in range(16):
                      filler.append(u_f1(fo))
                  for eo in range(4):
                      filler.append(u_f2(eo))
                  filler.append(u_st())

              qk_next = [None]
              for l in range(n_layers):
                if not static_weights and l == 0:
                    wk = wpool.tile([P, 4, D], FP16, tag="wk")
                    nc.sync.dma_start(wk, wk_dr.ap()[l].rearrange("(c p) n -> p c n", p=P))
                    wv = wpool.tile([P, 4, D], FP16, tag="wv")
                    nc.sync.dma_start(wv, wv_dr.ap()[l].rearrange("(c p) n -> p c n", p=P))
                    wo = wpool.tile([P, 4, D], FP16, tag="wo")
                    nc.sync.dma_start(wo, wo_dr.ap()[l].rearrange("(c p) n -> p c n", p=P))
                if not static_weights and l == 0:
                    w1 = wpool.tile([P, 4, FF], FP16, tag="w1")
                    nc.sync.dma_start(w1, w1_dr.ap()[l].rearrange("(c p) n -> p c n", p=P))
                    w2 = wpool.tile([P, 16, D], FP16, tag="w2")
                    nc.sync.dma_start(w2, w2_dr.ap()[l].rearrange("(c p) n -> p c n", p=P))

                for b in range(n_batch):
                    if skip_attn:
                        push_ffn(l, b, w1, w2)
                        continue
                    if qk_next[0] is None:
                        qk_cur = emit_qkproj(l, b)
                        emit_vproj(b, vts[b % 2])
                        flush_fin()
                    else:
                        qk_cur = qk_next[0]
                        qk_next[0] = None
                        if b == 0:
                            # LN2(l-1, b=3) finalize allocates from pp3; it
                            # must precede this layer's at-tile allocations
                            flush_fin()
                    hooks = [flush_fin, None, None]
                    if b + 1 < n_batch:
                        def _hk_qk(bn=b + 1):
                            qk_next[0] = emit_qkproj(l, bn)
                        def _hk_v(bn=b + 1):
                            emit_vproj(bn, vts[bn % 2])
                        hooks[1] = _hk_qk
                        hooks[2] = _hk_v
                    attnT = emit_attn(l, b, vts[b % 2], qk_cur, hooks, pull)
                    # out projection + bias(bo') + row-0 fix + residual
                    z1 = apool.tile([P, 4, S], FP16, tag="z", bufs=3)
                    for eo in range(4):
                        ps = pp1.tile([P, S], FP32, tag="ps1")
                        for ki in range(4):
                            nc.tensor.matmul(ps, wo[:, ki, P * eo:P * (eo + 1)],
                                             attnT[:, ki, :],
                                             start=(ki == 0), stop=(ki == 3))
                        nc.vector.scalar_tensor_tensor(
                            out=z1[:, eo, :], in0=ps,
                            scalar=cols[:, l, CI_BO + eo:CI_BO + eo + 1],
                            in1=xs[b][:, eo, :],
                            op0=ADD, op1=ADD)
                        nc.vector.tensor_tensor(
                            z1[:, eo, 0:1],
                            z1[:, eo, 0:1],
                            cols[:, l, CI_BVWO + eo:CI_BVWO + eo + 1], SUB)
                    pull(1800)
                    rows, rstd = layer_norm_stats(z1)
                    fins.append((b, make_ln_finalize(z1, xs[b], l, CI_G1,
                                                     CI_BT1, rows, rstd, pp1)))
                    if not skip_ffn:
                        push_ffn(l, b, w1, w2)

                prefetch_next = (not static_weights and l + 1 < n_layers)
                if prefetch_next and not skip_attn:
                    # next layer's attention weights; WAR-safe (this layer's
                    # qk/v/out projections are all emitted by now)
                    wk = wpool.tile([P, 4, D], FP16, tag="wk")
                    nc.sync.dma_start(wk, wk_dr.ap()[l + 1].rearrange("(c p) n -> p c n", p=P))
                    wv = wpool.tile([P, 4, D], FP16, tag="wv")
                    nc.sync.dma_start(wv, wv_dr.ap()[l + 1].rearrange("(c p) n -> p c n", p=P))
                    wo = wpool.tile([P, 4, D], FP16, tag="wo")
                    nc.sync.dma_start(wo, wo_dr.ap()[l + 1].rearrange("(c p) n -> p c n", p=P))
                # leftover FFN of this layer runs as a PE-dense block
                drain()
                if prefetch_next:
                    # w1/w2 for l+1 after the drain (WAR on this layer's
                    # readers); completes well before l+1's first f1 pull
                    w1 = wpool.tile([P, 4, FF], FP16, tag="w1")
                    nc.sync.dma_start(w1, w1_dr.ap()[l + 1].rearrange("(c p) n -> p c n", p=P))
                    w2 = wpool.tile([P, 16, D], FP16, tag="w2")
                    nc.sync.dma_start(w2, w2_dr.ap()[l + 1].rearrange("(c p) n -> p c n", p=P))
              flush_fin()

            for b in range(n_batch):
                nc.sync.dma_start(out_dr.ap()[b].rearrange("(c p) s -> p c s", p=P), xs[b])

    nc.compile()
    return nc


def _pos_emb():
    pos = np.arange(S, dtype=np.float32)[:, None]
    div = np.exp(np.arange(0, D, 2, dtype=np.float32) * (-np.log(10000.0) / D))
    pe = np.zeros((S, D), dtype=np.float32)
    pe[:, 0::2] = np.sin(pos * div)
    pe[:, 1::2] = np.cos(pos * div)
    return pe


def _pack_col(vec):
    """[512] -> [128, 4] feature-chunk columns."""
    return np.ascontiguousarray(vec.reshape(4, P).T)


def prepare_inputs(q_embed_data, qa_embed_data, boost_focus, Wk, bk, Wv, bv,
                   Wo, bo, ln1_g, ln1_b, W1, b1, W2, b2, ln2_g, ln2_b,
                   n_layers=L):
    """Host-side prep: pe add, transposes, bias folding, constant packing."""
    f16 = NP16
    pe = _pos_emb()
    x = (np.asarray(q_embed_data, np.float32) + pe[None]).transpose(0, 2, 1)
    y = (np.asarray(qa_embed_data, np.float32) + pe[None]).transpose(0, 2, 1)
    x = np.ascontiguousarray(x.astype(f16))
    y = np.ascontiguousarray(y.astype(f16))

    cexp = float((1.0 + float(np.asarray(boost_focus).reshape(-1)[0]))
                 / np.sqrt(DK))

    cols = np.zeros((P, n_layers, 48), np.float32)
    for l in range(n_layers):
        bo_eff = np.asarray(bv[l], np.float32) @ np.asarray(Wo[l], np.float32) \
            + np.asarray(bo[l], np.float32)
        bvwo = np.asarray(bv[l], np.float32) @ np.asarray(Wo[l], np.float32)
        cols[:, l, CI_BK:CI_BK + 4] = _pack_col(np.asarray(bk[l], np.float32))
        cols[:, l, CI_BO:CI_BO + 4] = _pack_col(bo_eff)
        cols[:, l, CI_BVWO:CI_BVWO + 4] = _pack_col(bvwo)
        cols[:, l, CI_B1:CI_B1 + 16] = np.asarray(b1[l], np.float32).reshape(16, P).T
        cols[:, l, CI_B2:CI_B2 + 4] = _pack_col(np.asarray(b2[l], np.float32))
        cols[:, l, CI_G1:CI_G1 + 4] = _pack_col(np.asarray(ln1_g[l], np.float32))
        cols[:, l, CI_BT1:CI_BT1 + 4] = _pack_col(np.asarray(ln1_b[l], np.float32))
        cols[:, l, CI_G2:CI_G2 + 4] = _pack_col(np.asarray(ln2_g[l], np.float32))
        cols[:, l, CI_BT2:CI_BT2 + 4] = _pack_col(np.asarray(ln2_b[l], np.float32))

    maskbf = np.triu(np.ones((P, P), np.float32), k=1).astype(f16)
    colh = np.stack([np.full(P, -1.0 / 512.0, np.float32),
                     np.full(P, 1.0 / 512.0, np.float32)], 1).astype(f16)
    rowr = np.zeros((1, 3, P), np.float32)
    rowr[0, 0, :] = 1.0
    rowr[0, 1, 0:DK] = 1.0
    rowr[0, 2, DK:P] = 1.0

    shared = {
        "wk": np.ascontiguousarray(np.asarray(Wk, np.float32)[:n_layers].astype(f16)),
        "wv": np.ascontiguousarray(np.asarray(Wv, np.float32)[:n_layers].astype(f16)),
        "wo": np.ascontiguousarray(np.asarray(Wo, np.float32)[:n_layers].astype(f16)),
        "w1": np.ascontiguousarray(np.asarray(W1, np.float32)[:n_layers].astype(f16)),
        "w2": np.ascontiguousarray(np.asarray(W2, np.float32)[:n_layers].astype(f16)),
        "cols": cols, "maskbf": maskbf, "colh": colh,
        "rowr": rowr.astype(f16),
        "epsc": np.full((1, 1), EPS, np.float32),
    }
    return x, y, shared, cexp


TRACE = False
LAST_RESULT = None


def kernel(**inputs):
    global LAST_RESULT
    x, y, shared, cexp = prepare_inputs(**inputs)
    nc = build_program(cexp)
    in_maps = []
    for core in range(NCORES):
        sl = slice(core * BLOC, (core + 1) * BLOC)
        m = dict(shared)
        m["x"] = np.ascontiguousarray(x[sl])
        m["y"] = np.ascontiguousarray(y[sl])
        in_maps.append(m)
    res = run_bass_kernel_spmd(nc, in_maps, core_ids=list(range(NCORES)),
                               trace=TRACE)
    LAST_RESULT = res
    out = np.concatenate([np.asarray(r["out"], np.float32) for r in res.results],
                         axis=0)
    return np.ascontiguousarray(out.transpose(0, 2, 1))
